# revision 50
# baseline (speedup 1.0000x reference)
"""Trainium2 Bass kernel: causal multi-head self-attention block (B=8, T=1024, E=768, H=12).

Sharding: data-parallel over batch — one batch element per NeuronCore, 8 cores,
no collectives. Each core computes the full attention block for its batch row.

Optimized schedule: consumption-ordered DMA stream (X first, W1 in column
blocks so each QK group unblocks after 0.4MB instead of 7MB), PE clock warmup
against the HAM throttle, replicated softmax denominator (64 ones-columns in
V -> column-bound ScalarE reciprocal, no gpsimd broadcast), score bursts
interleaved with qk/vproj/outproj matmuls (scores are ScalarE-EXP-paced),
and an incremental output projection whose last chunks interleave with the
final attention chunks.

Self-contained: hardcodes all shapes; only imports concourse (installed system-wide).
"""

import numpy as np

B, T, E, H, Dh = 8, 1024, 768, 12, 64
F3 = 3 * E            # 2304
KC = E // 128         # 6 e-chunks
MT = T // 128         # 8 t-tiles
NPAIR = H // 2        # 6 head pairs
SCALE = 1.0 / float(np.sqrt(Dh))

_NC_CACHE = None


def build_nc():
    import concourse.mybir as mybir
    from concourse import bacc
    from concourse.tile import TileContext
    from concourse.masks import make_identity

    bf = mybir.dt.bfloat16
    f32 = mybir.dt.float32
    COPY = mybir.ActivationFunctionType.Copy
    EXP = mybir.ActivationFunctionType.Exp
    LN = mybir.ActivationFunctionType.Ln
    ACT_SET_LN_EXP = 6  # natural_log_exp_and_others: holds Exp (+Ln fallback)

    nc = bacc.Bacc("TRN2", target_bir_lowering=False, debug=False, num_devices=B, name="attn_dp")

    X_ext = nc.declare_dram_parameter("X", [T, E], f32, isOutput=False)
    W1_ext = nc.declare_dram_parameter("W1", [E, F3], f32, isOutput=False)
    b1_ext = nc.declare_dram_parameter("b1", [F3], f32, isOutput=False)
    W2_ext = nc.declare_dram_parameter("W2", [E, E], f32, isOutput=False)
    b2_ext = nc.declare_dram_parameter("b2", [E], f32, isOutput=False)
    out_ext = nc.declare_dram_parameter("out", [T, E], bf, isOutput=True)

    with TileContext(nc) as tc:
        with (
            tc.tile_pool(name="persist", bufs=1) as persist,
            tc.tile_pool(name="stage", bufs=6) as stage,       # [128,768] f32: X, V, W2 chunks
            tc.tile_pool(name="w1stage", bufs=4) as w1stage,   # [128,6,128] f32: W1 QK m-blocks
            tc.tile_pool(name="ptpoolL", bufs=5) as ptpoolL,
            tc.tile_pool(name="ptpoolS", bufs=5) as ptpoolS,
            tc.tile_pool(name="opool", bufs=4) as opool,
            tc.tile_pool(name="denpool", bufs=2) as denpool,
            tc.tile_pool(name="ps_score", bufs=2, space="PSUM") as ps_score,
            tc.tile_pool(name="ps_z", bufs=2, space="PSUM") as ps_z,
            tc.tile_pool(name="ps_acc", bufs=2, space="PSUM") as ps_acc,
        ):
            # One activation-table load for the whole kernel (covers Exp + Copy).
            nc.scalar.add_instruction(mybir.InstLoadActFuncSet(
                name=nc.get_next_instruction_name(), ins=[], outs=[],
                act_func_set_id=ACT_SET_LN_EXP))

            # ---- input DMA stream (single sync queue; ordered by consumption) ----
            # X first (gates transposes and everything), then W1 QK column-blocks
            # in head-pair order (m, m+6), then V row-chunks, remaining QK blocks,
            # W2 row-chunks. Tiny bias DMAs go on the gpsimd queue in parallel.
            xst = []

            def x_dma(mt):
                t = stage.tile([128, E], f32, tag="stg")
                nc.sync.dma_start(out=t[:], in_=X_ext[mt * 128:(mt + 1) * 128, :])
                xst.append(t)

            w1qk_st = {}

            def w1qk_dma(m, eng=None):
                t = w1stage.tile([128, KC, 128], f32, tag="w1stg")
                (eng or nc.sync).dma_start(
                    out=t[:],
                    in_=W1_ext[:, m * 128:(m + 1) * 128].rearrange(
                        "(kc p) n -> p kc n", p=128))
                w1qk_st[m] = t

            vst = {}

            def v_dma(kc):
                t = stage.tile([128, E], f32, tag="stg")
                nc.sync.dma_start(
                    out=t[:], in_=W1_ext[kc * 128:(kc + 1) * 128, 1536:2304])
                vst[kc] = t

            w2st = {}

            def w2_dma(kc):
                t = stage.tile([128, E], f32, tag="stg")
                nc.sync.dma_start(out=t[:], in_=W2_ext[kc * 128:(kc + 1) * 128, :])
                w2st[kc] = t

            # DMA arrival order matches compute order: X rows 0:512, the first
            # QK weight blocks (m0, m6), X rows 512:1024, then m1/m7.
            for mt in range(4):
                x_dma(mt)
            for m in (0, 6):
                w1qk_dma(m)
            for mt in range(4, MT):
                x_dma(mt)
            for m in (1, 7):
                w1qk_dma(m)

            # ---- constants (gpsimd queue: parallel to the bulk stream) ----
            ident = persist.tile([128, 128], bf, tag="ident")
            make_identity(nc, ident[:])

            # PE warmup: the HAM clock gate keeps the PE at 1.2 GHz until it
            # sees ~3.4us of sustained matmul activity, and re-throttles after
            # ~3.4us idle. The ramp is DMA-paced (transposes cover only ~40%
            # of the X arrival window), so dummy matmuls woven into the FIFO
            # keep the clock at 2.4 GHz; they run only when real work waits.
            warm = ps_score.tile([128, 2, 512], f32, tag="score")

            def warm_fill(n):
                for _ in range(n):
                    nc.tensor.matmul(
                        warm[:, 0, 0:128], ident[:], ident[:],
                        start=True, stop=True, skip_group_check=True)

            warm_fill(32)
            # multiplicative causal mask for the diagonal 128x128 block:
            # mask[k, q] = 1 where q >= k else 0
            diagmask = persist.tile([128, 128], bf, tag="diagmask")
            nc.gpsimd.memset(diagmask[:], 1.0)
            nc.gpsimd.affine_select(
                out=diagmask[:], in_=diagmask[:],
                compare_op=mybir.AluOpType.is_ge, fill=0.0, base=0,
                pattern=[[1, 128]], channel_multiplier=-1,
            )

            # per-partition bias for Q/K part of b1: b1qk[p, m] = b1[m*128 + p]
            b1qk = persist.tile([128, 12], f32, tag="b1qk")
            nc.gpsimd.dma_start(
                out=b1qk[:], in_=b1_ext[0:1536].rearrange("(m p) -> p m", p=128)
            )
            # row biases, pre-broadcast across partitions
            b1v_f = stage.tile([1, E], f32, tag="stg")
            nc.gpsimd.dma_start(out=b1v_f[:], in_=b1_ext[None, 1536:2304])
            b1vb = persist.tile([128, E], f32, tag="b1vb")
            nc.gpsimd.partition_broadcast(b1vb[:], b1v_f[:])
            b2_f = stage.tile([1, E], f32, tag="stg")
            nc.gpsimd.dma_start(out=b2_f[:], in_=b2_ext[None, :])
            b2b = persist.tile([128, E], f32, tag="b2b")
            nc.gpsimd.partition_broadcast(b2b[:], b2_f[:])

            # ---- X: cast bf16 (ScalarE), per-block PE transpose to XT[e, t] ----
            XT = persist.tile([128, KC, T], bf, tag="XT")

            def xload(mt):
                xbf = stage.tile([128, E], bf, tag="xbf")
                nc.scalar.activation(xbf[:], xst[mt][:], COPY)
                for eg in range(KC // 2):  # pairs of e-chunks share one PSUM tile
                    pt_ps = ps_acc.tile([128, 256], bf, tag="acc")
                    nc.tensor.matmul(
                        pt_ps[:, 0:128], xbf[:, (2 * eg) * 128:(2 * eg + 1) * 128],
                        ident[:], is_transpose=True, start=True, stop=False,
                        skip_group_check=True)
                    nc.tensor.matmul(
                        pt_ps[:, 128:256], xbf[:, (2 * eg + 1) * 128:(2 * eg + 2) * 128],
                        ident[:], is_transpose=True, start=False, stop=True,
                        skip_group_check=True)
                    nc.vector.tensor_copy(
                        XT[:, 2 * eg:2 * eg + 2, mt * 128:(mt + 1) * 128],
                        pt_ps[:].rearrange("p (a n) -> p a n", a=2))

            # ---- W1 QK blocks: cast to bf16 on DVE as they arrive ----
            W1bf = persist.tile([128, KC, F3], bf, tag="W1bf")

            def w1qk_cast(m):
                nc.vector.tensor_copy(
                    W1bf[:, :, m * 128:(m + 1) * 128], w1qk_st[m][:])

            # QK[p, m, t]: m 0..5 = Q^T blocks (f rows m*128..), m 6..11 = K^T blocks
            QK = persist.tile([128, 12, T], bf, tag="QK")

            def qk_nh(m, nh):
                ps = ps_acc.tile([128, 512], f32, tag="acc")
                for kc in range(KC):
                    nc.tensor.matmul(
                        ps[:],
                        W1bf[:, kc, m * 128:(m + 1) * 128],
                        XT[:, kc, nh * 512:(nh + 1) * 512],
                        start=(kc == 0),
                        stop=(kc == KC - 1),
                    )
                # bias (per-partition) + cast to bf16
                nc.vector.tensor_scalar_add(
                    QK[:, m, nh * 512:(nh + 1) * 512], ps[:], b1qk[:, m:m + 1]
                )

            def qk_mtile(m):
                qk_nh(m, 0)
                qk_nh(m, 1)

            # interleave X transposes with the first QK projections: the nh=0
            # halves only need X rows 0:512, so QK work starts after 4 X tiles
            for mt in range(4):
                xload(mt)
                warm_fill(6)
            for m in (0, 6):
                w1qk_cast(m)
            warm_fill(6)
            qk_nh(0, 0)
            qk_nh(6, 0)
            for mt in range(4, MT):
                xload(mt)
                warm_fill(4)
            qk_nh(0, 1)
            qk_nh(6, 1)

            # ---- V_aug[t-part, kt, h, 0:64]=V, [64:128]=ones (denominator cols) ----
            # ones filled early on DVE (gpsimd scheduled these too late and
            # gated the first AV chunks)
            Vg = persist.tile([128, MT, H, 2 * Dh], bf, tag="Vg")
            for mt in range(MT):
                nc.vector.memset(Vg[:, mt, :, Dh:2 * Dh], 1.0)

            def vproj(mts):
                for kc in range(KC):
                    if (kc, 'cast') not in vst:
                        # split casts across engines so neither jams
                        if kc % 2 == 0:
                            nc.vector.tensor_copy(W1bf[:, kc, 1536:2304], vst[kc][:])
                        else:
                            nc.scalar.activation(W1bf[:, kc, 1536:2304], vst[kc][:], COPY)
                        vst[(kc, 'cast')] = True
                for mt in mts:
                    for g, (n0, n) in enumerate(((0, 512), (512, 256))):
                        ps = ps_acc.tile([128, 512], f32, tag="acc")
                        for kc in range(KC):
                            nc.tensor.matmul(
                                ps[:, 0:n],
                                XT[:, kc, mt * 128:(mt + 1) * 128],
                                W1bf[:, kc, 1536 + n0:1536 + n0 + n],
                                start=(kc == 0),
                                stop=(kc == KC - 1),
                            )
                        ng = n // Dh
                        # bias add (pre-broadcast row) + cast to bf16, strided into V_aug
                        nc.vector.tensor_add(
                            Vg[:, mt, 8 * g:8 * g + ng, 0:Dh],
                            ps[:, 0:n].rearrange("p (h d) -> p h d", d=Dh),
                            b1vb[:, n0:n0 + n].rearrange("p (h d) -> p h d", d=Dh),
                        )

            # ---- attention + Z^T (normalized) ----
            ZT = persist.tile([128, NPAIR, T], bf, tag="ZT")

            def scores_kt(hp, kt, pts):
                    L = T - kt * 128
                    # kt>=4 blocks span at most 512 q columns: use half-size
                    # tiles so the pools fit more buffers
                    if L > 512:
                        ptile = ptpoolL.tile([128, 2, 1024], bf, tag="ptL")
                    else:
                        ptile = ptpoolS.tile([128, 2, 512], bf, tag="ptS")
                    for c_off in range(0, L, 512):
                        n = min(512, L - c_off)
                        sc = ps_score.tile([128, 2, 512], f32, tag="score")
                        for h01 in range(2):
                            base = h01 * 64
                            nc.tensor.matmul(
                                sc[:, h01, 0:n],
                                QK[base:base + 64, 6 + hp, kt * 128:(kt + 1) * 128],
                                QK[base:base + 64, hp, kt * 128 + c_off:kt * 128 + c_off + n],
                                start=True,
                                stop=True,
                            )
                        nc.scalar.activation(
                            ptile[:, :, c_off:c_off + n], sc[:, :, 0:n], EXP, scale=SCALE)
                    # causal mask on the diagonal 128x128 block (cols 0:128 of this
                    # tile): multiply by the precomputed 0/1 mask (DVE, bf16 2x)
                    for h01 in range(2):
                        nc.vector.tensor_mul(
                            ptile[:, h01, 0:128], ptile[:, h01, 0:128], diagmask[:]
                        )
                    pts.append(ptile)

            def av_chunk(hp, q0, qn, pts):
                    for h01 in range(2):
                        z = ps_z.tile([128, 512], f32, tag="z")
                        h = 2 * hp + h01
                        kts = [kt for kt in range(MT) if kt * 128 < q0 + qn]
                        for kt in kts:
                            zoff = max(kt * 128 - q0, 0)
                            n = qn - zoff
                            poff = max(q0 - kt * 128, 0)
                            nc.tensor.matmul(
                                z[:, zoff:zoff + n],
                                Vg[:, kt, h, :],
                                pts[kt][:, h01, poff:poff + n],
                                start=(kt == kts[0]),
                                stop=(kt == kts[-1]),
                            )
                        # rows 64:128 hold the softmax denominator (replicated
                        # across 64 partitions by the ones-columns of V_aug).
                        # 1/den = exp(-ln(den)) on ScalarE — ACT cost is
                        # column-bound, so [64,qn] costs the same as [1,qn]
                        # and no partition broadcast is needed.
                        lnden = denpool.tile([64, 512], f32, tag="lnden")
                        nc.scalar.activation(lnden[:, 0:qn], z[Dh:2 * Dh, 0:qn], LN)
                        rec = denpool.tile([64, 512], bf, tag="rec")
                        nc.scalar.activation(rec[:, 0:qn], lnden[:, 0:qn], EXP, scale=-1.0)
                        nc.vector.tensor_mul(
                            ZT[h01 * 64:(h01 + 1) * 64, hp, q0:q0 + qn],
                            z[0:Dh, 0:qn],
                            rec[:, 0:qn],
                        )

            # ---- W2: cast on ScalarE (late; overlaps attention) ----
            W2bf = persist.tile([128, KC, E], bf, tag="W2bf")

            def w2_cast(kc):
                # gpsimd: idle after the ramp, keeps ScalarE free for EXP/recip
                nc.gpsimd.tensor_copy(W2bf[:, kc, :], w2st[kc][:])

            # ---- output projection, incremental split-K over head pairs ----
            outA = persist.tile([128, MT, E], bf, tag="outA")

            def outproj_acc(pcs, first):
                # accumulate sum over ZT pairs in `pcs` into outA (bf16 staging)
                for mt in range(MT):
                    for g, (n0, n) in enumerate(((0, 512), (512, 256))):
                        ps = ps_acc.tile([128, 512], f32, tag="acc")
                        for i, pc in enumerate(pcs):
                            nc.tensor.matmul(
                                ps[:, 0:n],
                                ZT[:, pc, mt * 128:(mt + 1) * 128],
                                W2bf[:, pc, n0:n0 + n],
                                start=(i == 0),
                                stop=(i == len(pcs) - 1),
                            )
                        if first:
                            # stage partial + bias (bf16)
                            nc.vector.tensor_add(
                                outA[:, mt, n0:n0 + n], ps[:, 0:n], b2b[:, n0:n0 + n])
                        else:
                            nc.vector.tensor_add(
                                outA[:, mt, n0:n0 + n], ps[:, 0:n], outA[:, mt, n0:n0 + n])

            def outproj_final(mts, pools):
                # deep-pipelined: both MMs of an mt issue before their STTs,
                # rotating over three PSUM pools (ps_score is free by now —
                # reuse its existing tag so no extra PSUM is allocated)
                i = 0
                for mt in mts:
                    pss = []
                    for g, (n0, n) in enumerate(((0, 512), (512, 256))):
                        pool = pools[i % len(pools)]
                        i += 1
                        if pool is ps_score:
                            pst = pool.tile([128, 2, 512], f32, tag="score")
                            ps = pst[:, 0, :]
                        elif pool is ps_acc:
                            pst = pool.tile([128, 512], f32, tag="acc")
                            ps = pst[:, :]
                        else:
                            pst = pool.tile([128, 512], f32, tag="z")
                            ps = pst[:, :]
                        pss.append(ps)
                        nc.tensor.matmul(
                            ps[:, 0:n],
                            ZT[:, 5, mt * 128:(mt + 1) * 128],
                            W2bf[:, 5, n0:n0 + n],
                            start=True,
                            stop=True,
                        )
                    for g, (n0, n) in enumerate(((0, 512), (512, 256))):
                        osb = opool.tile([128, 512], bf, tag="osb")
                        # osb = ps + outA  (bias already in outA)
                        nc.vector.scalar_tensor_tensor(
                            osb[:, 0:n], pss[g][:, 0:n], 1.0, outA[:, mt, n0:n0 + n],
                            op0=mybir.AluOpType.mult, op1=mybir.AluOpType.add)
                        # alternate queues so out-DMA transfers overlap
                        eng = nc.sync if g else nc.gpsimd
                        eng.dma_start(
                            out=out_ext[mt * 128:(mt + 1) * 128, n0:n0 + n],
                            in_=osb[:, 0:n])

            # the score bursts are EXP-paced (ps_score is double-buffered and
            # ScalarE drains ~1us/chunk): every hp interleaves its dense qk /
            # vproj / outproj matmuls between score chunks and av chunks so
            # the PE never idles behind a score chunk waiting on ScalarE.
            for hp in range(NPAIR):
                pts = []
                if hp == 0:
                    # sc0a chunks interleaved with the qk1 nh-halves
                    for kc in range(KC):
                        v_dma(kc)
                    w1qk_cast(1)
                    w1qk_cast(7)
                    scores_kt(hp, 0, pts)
                    qk_nh(1, 0)
                    scores_kt(hp, 1, pts)
                    qk_nh(1, 1)
                    scores_kt(hp, 2, pts)
                    qk_nh(7, 0)
                    scores_kt(hp, 3, pts)
                    qk_nh(7, 1)
                    scores_kt(hp, 4, pts)
                    scores_kt(hp, 5, pts)
                    vproj(range(0, 2))
                    scores_kt(hp, 6, pts)
                    vproj(range(2, 4))
                    scores_kt(hp, 7, pts)
                elif hp < NPAIR - 1:
                    # interleave score chunks (EXP-paced) with the next head
                    # pair's qk nh-halves (independent dense PE work) so the
                    # PE FIFO never blocks behind a ScalarE-gated score chunk
                    m = hp + 1
                    w1qk_cast(m)
                    w1qk_cast(m + 6)
                    scores_kt(hp, 0, pts)
                    qk_nh(m, 0)
                    scores_kt(hp, 1, pts)
                    qk_nh(m, 1)
                    scores_kt(hp, 2, pts)
                    qk_nh(m + 6, 0)
                    scores_kt(hp, 3, pts)
                    qk_nh(m + 6, 1)
                    scores_kt(hp, 4, pts)
                else:
                    for kt in range(4):
                        scores_kt(hp, kt, pts)
                    # pc3+pc4 accumulation fills the sc5a EXP trail
                    outproj_acc((3, 4), first=False)
                av_chunk(hp, 0, 512, pts)
                if hp == 0:
                    vproj(range(4, MT))
                else:
                    for kt in range(4 if hp == NPAIR - 1 else 5, MT):
                        scores_kt(hp, kt, pts)
                if hp == 0:
                    # remaining QK W1 column-blocks + W2 row-chunks
                    for m in (2, 8, 3, 9, 4, 10, 5, 11):
                        w1qk_dma(m)
                    for kc in range(KC):
                        w2_dma(kc)
                if hp == NPAIR - 1:
                    # ZT[:,5] for q<512 is complete after av(5,0): finish those
                    # output rows while av(5,1) waits on its EXPs, and
                    # interleave the last outputs with 256-wide av chunks
                    outproj_final(range(0, 4), (ps_score, ps_acc))
                    av_chunk(hp, 512, 256, pts)
                    outproj_final(range(4, 6), (ps_score, ps_acc))
                    av_chunk(hp, 768, 256, pts)
                    outproj_final(range(6, MT), (ps_score, ps_acc, ps_z))
                else:
                    av_chunk(hp, 512, 512, pts)
                if hp == 2:
                    for kc in range(KC):
                        w2_cast(kc)
                    outproj_acc((0, 1, 2), first=True)

    nc.compile()
    return nc


def _get_nc():
    global _NC_CACHE
    if _NC_CACHE is None:
        _NC_CACHE = build_nc()
    return _NC_CACHE


def _in_maps(X, W1, b1, W2, b2):
    X = np.ascontiguousarray(np.asarray(X, dtype=np.float32))
    W1 = np.ascontiguousarray(np.asarray(W1, dtype=np.float32))
    b1 = np.ascontiguousarray(np.asarray(b1, dtype=np.float32))
    W2 = np.ascontiguousarray(np.asarray(W2, dtype=np.float32))
    b2 = np.ascontiguousarray(np.asarray(b2, dtype=np.float32))
    assert X.shape == (B, T, E)
    return [
        {"X": X[i], "W1": W1, "b1": b1, "W2": W2, "b2": b2}
        for i in range(B)
    ]


def kernel(X, W1, b1, W2, b2):
    from concourse.bass_utils import run_bass_kernel_spmd

    nc = _get_nc()
    res = run_bass_kernel_spmd(nc, _in_maps(X, W1, b1, W2, b2), core_ids=list(range(B)))
    return np.stack([res.results[i]["out"] for i in range(B)]).astype(np.float32)


def kernel_traced(X, W1, b1, W2, b2, tmpdir=None):
    """Like kernel() but with neuron-profile tracing; returns (out, BassKernelResults)."""
    from concourse.bass_utils import run_bass_kernel_spmd

    nc = _get_nc()
    res = run_bass_kernel_spmd(
        nc, _in_maps(X, W1, b1, W2, b2), core_ids=list(range(B)),
        trace=True, tmpdir=tmpdir,
    )
    out = np.stack([res.results[i]["out"] for i in range(B)]).astype(np.float32)
    return out, res


# revision 51
# speedup vs baseline: 1.0052x; 1.0052x over previous
"""Trainium2 Bass kernel: causal multi-head self-attention block (B=8, T=1024, E=768, H=12).

Sharding: data-parallel over batch — one batch element per NeuronCore, 8 cores,
no collectives. Each core computes the full attention block for its batch row.

Optimized schedule: consumption-ordered DMA stream (X first, W1 in column
blocks so each QK group unblocks after 0.4MB instead of 7MB), PE clock warmup
against the HAM throttle, replicated softmax denominator (64 ones-columns in
V -> column-bound ScalarE reciprocal, no gpsimd broadcast), score bursts
interleaved with qk/vproj/outproj matmuls (scores are ScalarE-EXP-paced),
and an incremental output projection whose last chunks interleave with the
final attention chunks.

Self-contained: hardcodes all shapes; only imports concourse (installed system-wide).
"""

import numpy as np

B, T, E, H, Dh = 8, 1024, 768, 12, 64
F3 = 3 * E            # 2304
KC = E // 128         # 6 e-chunks
MT = T // 128         # 8 t-tiles
NPAIR = H // 2        # 6 head pairs
SCALE = 1.0 / float(np.sqrt(Dh))

_NC_CACHE = None


def build_nc():
    import concourse.mybir as mybir
    from concourse import bacc
    from concourse.tile import TileContext
    from concourse.masks import make_identity

    bf = mybir.dt.bfloat16
    f32 = mybir.dt.float32
    COPY = mybir.ActivationFunctionType.Copy
    EXP = mybir.ActivationFunctionType.Exp
    LN = mybir.ActivationFunctionType.Ln
    ACT_SET_LN_EXP = 6  # natural_log_exp_and_others: holds Exp (+Ln fallback)

    nc = bacc.Bacc("TRN2", target_bir_lowering=False, debug=False, num_devices=B, name="attn_dp")

    X_ext = nc.declare_dram_parameter("X", [T, E], f32, isOutput=False)
    W1_ext = nc.declare_dram_parameter("W1", [E, F3], f32, isOutput=False)
    b1_ext = nc.declare_dram_parameter("b1", [F3], f32, isOutput=False)
    W2_ext = nc.declare_dram_parameter("W2", [E, E], f32, isOutput=False)
    b2_ext = nc.declare_dram_parameter("b2", [E], f32, isOutput=False)
    out_ext = nc.declare_dram_parameter("out", [T, E], bf, isOutput=True)

    with TileContext(nc) as tc:
        with (
            tc.tile_pool(name="persist", bufs=1) as persist,
            tc.tile_pool(name="stage", bufs=6) as stage,       # [128,768] f32: X, V, W2 chunks
            tc.tile_pool(name="w1stage", bufs=4) as w1stage,   # [128,6,128] f32: W1 QK m-blocks
            tc.tile_pool(name="ptpoolL", bufs=5) as ptpoolL,
            tc.tile_pool(name="ptpoolS", bufs=5) as ptpoolS,
            tc.tile_pool(name="opool", bufs=3) as opool,
            tc.tile_pool(name="denpool", bufs=2) as denpool,
            tc.tile_pool(name="ps_score", bufs=2, space="PSUM") as ps_score,
            tc.tile_pool(name="ps_z", bufs=2, space="PSUM") as ps_z,
            tc.tile_pool(name="ps_acc", bufs=2, space="PSUM") as ps_acc,
        ):
            # One activation-table load for the whole kernel (covers Exp + Copy).
            nc.scalar.add_instruction(mybir.InstLoadActFuncSet(
                name=nc.get_next_instruction_name(), ins=[], outs=[],
                act_func_set_id=ACT_SET_LN_EXP))

            # ---- input DMA stream (single sync queue; ordered by consumption) ----
            # X first (gates transposes and everything), then W1 QK column-blocks
            # in head-pair order (m, m+6), then V row-chunks, remaining QK blocks,
            # W2 row-chunks. Tiny bias DMAs go on the gpsimd queue in parallel.
            xst = []

            def x_dma(mt):
                t = stage.tile([128, E], f32, tag="stg")
                nc.sync.dma_start(out=t[:], in_=X_ext[mt * 128:(mt + 1) * 128, :])
                xst.append(t)

            w1qk_st = {}

            def w1qk_dma(m, eng=None):
                t = w1stage.tile([128, KC, 128], f32, tag="w1stg")
                (eng or nc.sync).dma_start(
                    out=t[:],
                    in_=W1_ext[:, m * 128:(m + 1) * 128].rearrange(
                        "(kc p) n -> p kc n", p=128))
                w1qk_st[m] = t

            vst = {}

            def v_dma(kc):
                t = stage.tile([128, E], f32, tag="stg")
                nc.sync.dma_start(
                    out=t[:], in_=W1_ext[kc * 128:(kc + 1) * 128, 1536:2304])
                vst[kc] = t

            w2st = {}

            def w2_dma(kc):
                t = stage.tile([128, E], f32, tag="stg")
                nc.sync.dma_start(out=t[:], in_=W2_ext[kc * 128:(kc + 1) * 128, :])
                w2st[kc] = t

            # DMA arrival order matches compute order: X rows 0:512, the first
            # QK weight blocks (m0, m6), X rows 512:1024, then m1/m7.
            for mt in range(4):
                x_dma(mt)
            for m in (0, 6):
                w1qk_dma(m)
            for mt in range(4, MT):
                x_dma(mt)
            for m in (1, 7):
                w1qk_dma(m)

            # ---- constants (gpsimd queue: parallel to the bulk stream) ----
            ident = persist.tile([128, 128], bf, tag="ident")
            make_identity(nc, ident[:])

            # PE warmup: the HAM clock gate keeps the PE at 1.2 GHz until it
            # sees ~3.4us of sustained matmul activity, and re-throttles after
            # ~3.4us idle. The ramp is DMA-paced (transposes cover only ~40%
            # of the X arrival window), so dummy matmuls woven into the FIFO
            # keep the clock at 2.4 GHz; they run only when real work waits.
            warm = ps_score.tile([128, 2, 512], f32, tag="score")

            def warm_fill(n):
                for _ in range(n):
                    nc.tensor.matmul(
                        warm[:, 0, 0:128], ident[:], ident[:],
                        start=True, stop=True, skip_group_check=True)

            warm_fill(32)
            # multiplicative causal mask for the diagonal 128x128 block:
            # mask[k, q] = 1 where q >= k else 0
            diagmask = persist.tile([128, 128], bf, tag="diagmask")
            nc.gpsimd.memset(diagmask[:], 1.0)
            nc.gpsimd.affine_select(
                out=diagmask[:], in_=diagmask[:],
                compare_op=mybir.AluOpType.is_ge, fill=0.0, base=0,
                pattern=[[1, 128]], channel_multiplier=-1,
            )

            # per-partition bias for Q/K part of b1: b1qk[p, m] = b1[m*128 + p]
            b1qk = persist.tile([128, 12], f32, tag="b1qk")
            nc.gpsimd.dma_start(
                out=b1qk[:], in_=b1_ext[0:1536].rearrange("(m p) -> p m", p=128)
            )
            # row biases, pre-broadcast across partitions
            b1v_f = stage.tile([1, E], f32, tag="stg")
            nc.gpsimd.dma_start(out=b1v_f[:], in_=b1_ext[None, 1536:2304])
            b1vb = persist.tile([128, E], f32, tag="b1vb")
            nc.gpsimd.partition_broadcast(b1vb[:], b1v_f[:])
            b2_f = stage.tile([1, E], f32, tag="stg")
            nc.gpsimd.dma_start(out=b2_f[:], in_=b2_ext[None, :])
            b2b = persist.tile([128, E], f32, tag="b2b")
            nc.gpsimd.partition_broadcast(b2b[:], b2_f[:])

            # ---- X: cast bf16 (ScalarE), per-block PE transpose to XT[e, t] ----
            XT = persist.tile([128, KC, T], bf, tag="XT")

            def xload(mt):
                xbf = stage.tile([128, E], bf, tag="xbf")
                nc.scalar.activation(xbf[:], xst[mt][:], COPY)
                for eg in range(KC // 2):  # pairs of e-chunks share one PSUM tile
                    pt_ps = ps_acc.tile([128, 256], bf, tag="acc")
                    nc.tensor.matmul(
                        pt_ps[:, 0:128], xbf[:, (2 * eg) * 128:(2 * eg + 1) * 128],
                        ident[:], is_transpose=True, start=True, stop=False,
                        skip_group_check=True)
                    nc.tensor.matmul(
                        pt_ps[:, 128:256], xbf[:, (2 * eg + 1) * 128:(2 * eg + 2) * 128],
                        ident[:], is_transpose=True, start=False, stop=True,
                        skip_group_check=True)
                    nc.vector.tensor_copy(
                        XT[:, 2 * eg:2 * eg + 2, mt * 128:(mt + 1) * 128],
                        pt_ps[:].rearrange("p (a n) -> p a n", a=2))

            # ---- W1 QK blocks: cast to bf16 on DVE as they arrive ----
            W1bf = persist.tile([128, KC, F3], bf, tag="W1bf")

            def w1qk_cast(m):
                nc.vector.tensor_copy(
                    W1bf[:, :, m * 128:(m + 1) * 128], w1qk_st[m][:])

            # QK[p, m, t]: m 0..5 = Q^T blocks (f rows m*128..), m 6..11 = K^T blocks
            QK = persist.tile([128, 12, T], bf, tag="QK")

            def qk_nh(m, nh):
                ps = ps_acc.tile([128, 512], f32, tag="acc")
                for kc in range(KC):
                    nc.tensor.matmul(
                        ps[:],
                        W1bf[:, kc, m * 128:(m + 1) * 128],
                        XT[:, kc, nh * 512:(nh + 1) * 512],
                        start=(kc == 0),
                        stop=(kc == KC - 1),
                    )
                # bias (per-partition) + cast to bf16
                nc.vector.tensor_scalar_add(
                    QK[:, m, nh * 512:(nh + 1) * 512], ps[:], b1qk[:, m:m + 1]
                )

            def qk_mtile(m):
                qk_nh(m, 0)
                qk_nh(m, 1)

            # interleave X transposes with the first QK projections: the nh=0
            # halves only need X rows 0:512, so QK work starts after 4 X tiles
            for mt in range(4):
                xload(mt)
                warm_fill(6)
            for m in (0, 6):
                w1qk_cast(m)
            warm_fill(6)
            qk_nh(0, 0)
            qk_nh(6, 0)
            for mt in range(4, MT):
                xload(mt)
                warm_fill(4)
            qk_nh(0, 1)
            qk_nh(6, 1)

            # ---- V_aug[t-part, kt, h, 0:64]=V, [64:128]=ones (denominator cols) ----
            # ones filled early on DVE (gpsimd scheduled these too late and
            # gated the first AV chunks)
            Vg = persist.tile([128, MT, H, 2 * Dh], bf, tag="Vg")
            for mt in range(MT):
                nc.vector.memset(Vg[:, mt, :, Dh:2 * Dh], 1.0)

            def vproj(mts):
                for kc in range(KC):
                    if (kc, 'cast') not in vst:
                        # split casts across engines so neither jams
                        if kc % 2 == 0:
                            nc.vector.tensor_copy(W1bf[:, kc, 1536:2304], vst[kc][:])
                        else:
                            nc.scalar.activation(W1bf[:, kc, 1536:2304], vst[kc][:], COPY)
                        vst[(kc, 'cast')] = True
                for mt in mts:
                    for g, (n0, n) in enumerate(((0, 512), (512, 256))):
                        ps = ps_acc.tile([128, 512], f32, tag="acc")
                        for kc in range(KC):
                            nc.tensor.matmul(
                                ps[:, 0:n],
                                XT[:, kc, mt * 128:(mt + 1) * 128],
                                W1bf[:, kc, 1536 + n0:1536 + n0 + n],
                                start=(kc == 0),
                                stop=(kc == KC - 1),
                            )
                        ng = n // Dh
                        # bias add (pre-broadcast row) + cast to bf16, strided into V_aug
                        nc.vector.tensor_add(
                            Vg[:, mt, 8 * g:8 * g + ng, 0:Dh],
                            ps[:, 0:n].rearrange("p (h d) -> p h d", d=Dh),
                            b1vb[:, n0:n0 + n].rearrange("p (h d) -> p h d", d=Dh),
                        )

            # ---- attention + Z^T (normalized) ----
            ZT = persist.tile([128, NPAIR, T], bf, tag="ZT")

            def scores_kt(hp, kt, pts):
                    L = T - kt * 128
                    # kt>=4 blocks span at most 512 q columns: use half-size
                    # tiles so the pools fit more buffers
                    if L > 512:
                        ptile = ptpoolL.tile([128, 2, 1024], bf, tag="ptL")
                    else:
                        ptile = ptpoolS.tile([128, 2, 512], bf, tag="ptS")
                    for c_off in range(0, L, 512):
                        n = min(512, L - c_off)
                        sc = ps_score.tile([128, 2, 512], f32, tag="score")
                        for h01 in range(2):
                            base = h01 * 64
                            nc.tensor.matmul(
                                sc[:, h01, 0:n],
                                QK[base:base + 64, 6 + hp, kt * 128:(kt + 1) * 128],
                                QK[base:base + 64, hp, kt * 128 + c_off:kt * 128 + c_off + n],
                                start=True,
                                stop=True,
                            )
                        nc.scalar.activation(
                            ptile[:, :, c_off:c_off + n], sc[:, :, 0:n], EXP, scale=SCALE)
                    # causal mask on the diagonal 128x128 block (cols 0:128 of this
                    # tile): multiply by the precomputed 0/1 mask (DVE, bf16 2x)
                    for h01 in range(2):
                        nc.vector.tensor_mul(
                            ptile[:, h01, 0:128], ptile[:, h01, 0:128], diagmask[:]
                        )
                    pts.append(ptile)

            def av_chunk(hp, q0, qn, pts):
                    for h01 in range(2):
                        z = ps_z.tile([128, 512], f32, tag="z")
                        h = 2 * hp + h01
                        kts = [kt for kt in range(MT) if kt * 128 < q0 + qn]
                        for kt in kts:
                            zoff = max(kt * 128 - q0, 0)
                            n = qn - zoff
                            poff = max(q0 - kt * 128, 0)
                            nc.tensor.matmul(
                                z[:, zoff:zoff + n],
                                Vg[:, kt, h, :],
                                pts[kt][:, h01, poff:poff + n],
                                start=(kt == kts[0]),
                                stop=(kt == kts[-1]),
                            )
                        # rows 64:128 hold the softmax denominator (replicated
                        # across 64 partitions by the ones-columns of V_aug).
                        # 1/den = exp(-ln(den)) on ScalarE — ACT cost is
                        # column-bound, so [64,qn] costs the same as [1,qn]
                        # and no partition broadcast is needed.
                        lnden = denpool.tile([64, 512], f32, tag="lnden")
                        nc.scalar.activation(lnden[:, 0:qn], z[Dh:2 * Dh, 0:qn], LN)
                        rec = denpool.tile([64, 512], bf, tag="rec")
                        nc.scalar.activation(rec[:, 0:qn], lnden[:, 0:qn], EXP, scale=-1.0)
                        nc.vector.tensor_mul(
                            ZT[h01 * 64:(h01 + 1) * 64, hp, q0:q0 + qn],
                            z[0:Dh, 0:qn],
                            rec[:, 0:qn],
                        )

            # ---- W2: cast on ScalarE (late; overlaps attention) ----
            W2bf = persist.tile([128, KC, E], bf, tag="W2bf")

            def w2_cast(kc):
                # gpsimd: idle after the ramp, keeps ScalarE free for EXP/recip
                nc.gpsimd.tensor_copy(W2bf[:, kc, :], w2st[kc][:])

            # ---- output projection, incremental split-K over head pairs ----
            outA = persist.tile([128, MT, E], bf, tag="outA")

            def outproj_acc(pcs, first):
                # accumulate sum over ZT pairs in `pcs` into outA (bf16 staging)
                for mt in range(MT):
                    for g, (n0, n) in enumerate(((0, 512), (512, 256))):
                        ps = ps_acc.tile([128, 512], f32, tag="acc")
                        for i, pc in enumerate(pcs):
                            nc.tensor.matmul(
                                ps[:, 0:n],
                                ZT[:, pc, mt * 128:(mt + 1) * 128],
                                W2bf[:, pc, n0:n0 + n],
                                start=(i == 0),
                                stop=(i == len(pcs) - 1),
                            )
                        if first:
                            # stage partial + bias (bf16)
                            nc.vector.tensor_add(
                                outA[:, mt, n0:n0 + n], ps[:, 0:n], b2b[:, n0:n0 + n])
                        else:
                            nc.vector.tensor_add(
                                outA[:, mt, n0:n0 + n], ps[:, 0:n], outA[:, mt, n0:n0 + n])

            def outproj_final(mts, pools):
                # deep-pipelined: both MMs of an mt issue before their STTs,
                # rotating over three PSUM pools (ps_score is free by now —
                # reuse its existing tag so no extra PSUM is allocated)
                i = 0
                for mt in mts:
                    pss = []
                    for g, (n0, n) in enumerate(((0, 512), (512, 256))):
                        pool = pools[i % len(pools)]
                        i += 1
                        if pool is ps_score:
                            pst = pool.tile([128, 2, 512], f32, tag="score")
                            ps = pst[:, 0, :]
                        elif pool is ps_acc:
                            pst = pool.tile([128, 512], f32, tag="acc")
                            ps = pst[:, :]
                        else:
                            pst = pool.tile([128, 512], f32, tag="z")
                            ps = pst[:, :]
                        pss.append(ps)
                        nc.tensor.matmul(
                            ps[:, 0:n],
                            ZT[:, 5, mt * 128:(mt + 1) * 128],
                            W2bf[:, 5, n0:n0 + n],
                            start=True,
                            stop=True,
                        )
                    for g, (n0, n) in enumerate(((0, 512), (512, 256))):
                        osb = opool.tile([128, 512], bf, tag="osb")
                        # osb = ps + outA  (bias already in outA)
                        nc.vector.scalar_tensor_tensor(
                            osb[:, 0:n], pss[g][:, 0:n], 1.0, outA[:, mt, n0:n0 + n],
                            op0=mybir.AluOpType.mult, op1=mybir.AluOpType.add)
                        # alternate queues so out-DMA transfers overlap
                        eng = nc.sync if g else nc.gpsimd
                        eng.dma_start(
                            out=out_ext[mt * 128:(mt + 1) * 128, n0:n0 + n],
                            in_=osb[:, 0:n])

            # the score bursts are EXP-paced (ps_score is double-buffered and
            # ScalarE drains ~1us/chunk): every hp interleaves its dense qk /
            # vproj / outproj matmuls between score chunks and av chunks so
            # the PE never idles behind a score chunk waiting on ScalarE.
            for hp in range(NPAIR):
                pts = []
                if hp == 0:
                    # sc0a chunks interleaved with the qk1 nh-halves
                    for kc in range(KC):
                        v_dma(kc)
                    w1qk_cast(1)
                    w1qk_cast(7)
                    scores_kt(hp, 0, pts)
                    qk_nh(1, 0)
                    scores_kt(hp, 1, pts)
                    qk_nh(1, 1)
                    scores_kt(hp, 2, pts)
                    qk_nh(7, 0)
                    scores_kt(hp, 3, pts)
                    qk_nh(7, 1)
                    scores_kt(hp, 4, pts)
                    scores_kt(hp, 5, pts)
                    vproj(range(0, 2))
                    scores_kt(hp, 6, pts)
                    vproj(range(2, 4))
                    scores_kt(hp, 7, pts)
                elif hp < NPAIR - 1:
                    # interleave score chunks (EXP-paced) with the next head
                    # pair's qk nh-halves (independent dense PE work) so the
                    # PE FIFO never blocks behind a ScalarE-gated score chunk
                    m = hp + 1
                    w1qk_cast(m)
                    w1qk_cast(m + 6)
                    scores_kt(hp, 0, pts)
                    qk_nh(m, 0)
                    scores_kt(hp, 1, pts)
                    qk_nh(m, 1)
                    scores_kt(hp, 2, pts)
                    qk_nh(m + 6, 0)
                    scores_kt(hp, 3, pts)
                    qk_nh(m + 6, 1)
                    scores_kt(hp, 4, pts)
                else:
                    for kt in range(4):
                        scores_kt(hp, kt, pts)
                    # pc3+pc4 accumulation fills the sc5a EXP trail
                    outproj_acc((3, 4), first=False)
                av_chunk(hp, 0, 512, pts)
                if hp == 0:
                    vproj(range(4, MT))
                else:
                    for kt in range(4 if hp == NPAIR - 1 else 5, MT):
                        scores_kt(hp, kt, pts)
                if hp == 0:
                    # remaining QK W1 column-blocks + W2 row-chunks
                    for m in (2, 8, 3, 9, 4, 10, 5, 11):
                        w1qk_dma(m)
                    for kc in range(KC):
                        w2_dma(kc)
                if hp == NPAIR - 1:
                    # ZT[:,5] for q<512 is complete after av(5,0): finish those
                    # output rows while av(5,1) waits on its EXPs, and
                    # interleave the last outputs with 256-wide av chunks
                    outproj_final(range(0, 4), (ps_score, ps_acc))
                    av_chunk(hp, 512, 256, pts)
                    outproj_final(range(4, 6), (ps_score, ps_acc))
                    av_chunk(hp, 768, 256, pts)
                    outproj_final(range(6, MT), (ps_score, ps_acc, ps_z))
                else:
                    av_chunk(hp, 512, 512, pts)
                if hp == 2:
                    for kc in range(KC):
                        w2_cast(kc)
                    outproj_acc((0, 1, 2), first=True)

    nc.compile()
    return nc


def _get_nc():
    global _NC_CACHE
    if _NC_CACHE is None:
        _NC_CACHE = build_nc()
    return _NC_CACHE


def _in_maps(X, W1, b1, W2, b2):
    X = np.ascontiguousarray(np.asarray(X, dtype=np.float32))
    W1 = np.ascontiguousarray(np.asarray(W1, dtype=np.float32))
    b1 = np.ascontiguousarray(np.asarray(b1, dtype=np.float32))
    W2 = np.ascontiguousarray(np.asarray(W2, dtype=np.float32))
    b2 = np.ascontiguousarray(np.asarray(b2, dtype=np.float32))
    assert X.shape == (B, T, E)
    return [
        {"X": X[i], "W1": W1, "b1": b1, "W2": W2, "b2": b2}
        for i in range(B)
    ]


def kernel(X, W1, b1, W2, b2):
    from concourse.bass_utils import run_bass_kernel_spmd

    nc = _get_nc()
    res = run_bass_kernel_spmd(nc, _in_maps(X, W1, b1, W2, b2), core_ids=list(range(B)))
    return np.stack([res.results[i]["out"] for i in range(B)]).astype(np.float32)


def kernel_traced(X, W1, b1, W2, b2, tmpdir=None):
    """Like kernel() but with neuron-profile tracing; returns (out, BassKernelResults)."""
    from concourse.bass_utils import run_bass_kernel_spmd

    nc = _get_nc()
    res = run_bass_kernel_spmd(
        nc, _in_maps(X, W1, b1, W2, b2), core_ids=list(range(B)),
        trace=True, tmpdir=tmpdir,
    )
    out = np.stack([res.results[i]["out"] for i in range(B)]).astype(np.float32)
    return out, res
